# revision 5
# baseline (speedup 1.0000x reference)
"""Causal single-head attention (B=4, S=4096, D=1024) on 8 trn2 NeuronCores.

Sharding: 2 cores per batch element. Each core owns 16 interleaved 128-row
query blocks (core parity k takes global blocks g = 2t + k, t = 0..15), which
balances the causal triangle exactly: local block t attends to key columns
[0, (2t+2)*128), identical extent on every core, so one SPMD program serves
all 8 cores. The causal boundary only affects the last 256 key columns of
each block's extent; a per-core constant additive mask [128, 256] handles it.

Per core, on device (all matmuls bf16 with fp32 PSUM accumulation):
  qT[o,m] / kT[o,s] via W^T-stationary matmuls, v[s,o] via x^T-stationary,
  scores = qT^T @ kT (per 512-col group), +mask, exp(x/32) on ACT with
  row-sum accumulation, PE-transpose of attn blocks, attn^T-stationary PV
  matmuls, then a per-row 1/sum rescale fused into the PSUM->SBUF eviction.
"""

import math

import numpy as np
import ml_dtypes

from concourse import bacc, mybir, tile
from concourse.bass_utils import run_bass_kernel_spmd

B, S, D = 4, 4096, 1024
NCORES = 8
P = 128
DK = D // P          # 8 contraction chunks of 128 over d_in / d_out
NQB = (S // 2) // P  # 16 local query blocks per core
NEG = -1.0e30

_CACHE = {}


def _build_program():
    bf16 = mybir.dt.bfloat16
    f32 = mybir.dt.float32
    nc = bacc.Bacc(
        "TRN2",
        target_bir_lowering=False,
        debug=False,
        num_devices=NCORES,
    )

    xT_d = nc.dram_tensor("xT", [DK, P, S], bf16, kind="ExternalInput")
    xTq_d = nc.dram_tensor("xTq", [DK, P, S // 2], bf16, kind="ExternalInput")
    wqT_d = nc.dram_tensor("wqT", [DK, P, D], bf16, kind="ExternalInput")
    wkT_d = nc.dram_tensor("wkT", [DK, P, D], bf16, kind="ExternalInput")
    wvT_d = nc.dram_tensor("wvT", [DK, P, D], bf16, kind="ExternalInput")
    mask_d = nc.dram_tensor("mask", [P, 2 * P], f32, kind="ExternalInput")
    ident_d = nc.dram_tensor("ident", [P, P], bf16, kind="ExternalInput")
    out_d = nc.dram_tensor("out", [NQB, P, D], f32, kind="ExternalOutput")

    with tile.TileContext(nc) as tc:
        with (
            tc.tile_pool(name="const", bufs=1) as constp,
            tc.tile_pool(name="w", bufs=9) as wp,
            tc.tile_pool(name="slab", bufs=12) as slabp,
            tc.tile_pool(name="qT", bufs=1) as qTp,
            tc.tile_pool(name="kT", bufs=1) as kTp,
            tc.tile_pool(name="v", bufs=1) as vp,
            tc.tile_pool(name="attn", bufs=3) as attnp,
            tc.tile_pool(name="attnT", bufs=1) as attnTp,
            tc.tile_pool(name="stat", bufs=2) as statp,
            tc.tile_pool(name="outst", bufs=1) as outp,
            tc.tile_pool(name="psmm", bufs=3, space="PSUM") as psmm,
            tc.tile_pool(name="pstr", bufs=2, space="PSUM") as pstr,
            tc.tile_pool(name="pspv", bufs=3, space="PSUM") as pspv,
        ):
            mask_t = constp.tile([P, 2 * P], f32, tag="mask", name="mask_t")
            nc.sync.dma_start(mask_t[:], mask_d[:])
            ident_t = constp.tile([P, P], bf16, tag="ident", name="ident_t")
            nc.sync.dma_start(ident_t[:], ident_d[:])

            def load_w(wdram):
                ws = []
                for d in range(DK):
                    w = wp.tile([P, D], bf16, tag="w", name=f"w{d}")
                    nc.sync.dma_start(w[:], wdram[d])
                    ws.append(w)
                return ws

            def load_slab(src, c0, cw):
                slab = []
                for d in range(DK):
                    t = slabp.tile([P, 512], bf16, tag="slab", name=f"slab{d}")
                    nc.sync.dma_start(t[:, :cw], src[d][:, c0 : c0 + cw])
                    slab.append(t)
                return slab

            # ---- Q projection: qT[o, m] for 2048 local query rows
            wq = load_w(wqT_d)
            qT = [qTp.tile([P, S // 2], bf16, tag=f"qT{o}", name=f"qT{o}") for o in range(DK)]
            for mg in range(4):
                slab = load_slab(xTq_d, mg * 512, 512)
                for o in range(DK):
                    ps = psmm.tile([P, 512], f32, tag="psmm", name="ps")
                    for d in range(DK):
                        nc.tensor.matmul(
                            ps[:],
                            wq[d][:, o * P : (o + 1) * P],
                            slab[d][:],
                            start=(d == 0),
                            stop=(d == DK - 1),
                        )
                    nc.vector.tensor_copy(qT[o][:, mg * 512 : (mg + 1) * 512], ps[:])

            # ---- K projection: kT[o, s] for all 4096 rows
            wk = load_w(wkT_d)
            kT = [kTp.tile([P, S], bf16, tag=f"kT{o}", name=f"kT{o}") for o in range(DK)]
            for sg in range(8):
                slab = load_slab(xT_d, sg * 512, 512)
                for o in range(DK):
                    ps = psmm.tile([P, 512], f32, tag="psmm", name="ps")
                    for d in range(DK):
                        nc.tensor.matmul(
                            ps[:],
                            wk[d][:, o * P : (o + 1) * P],
                            slab[d][:],
                            start=(d == 0),
                            stop=(d == DK - 1),
                        )
                    nc.vector.tensor_copy(kT[o][:, sg * 512 : (sg + 1) * 512], ps[:])

            # ---- V projection: v[s, o] for all 4096 rows
            wv = load_w(wvT_d)
            v = [vp.tile([P, D], bf16, tag=f"v{j}", name=f"v{j}") for j in range(S // P)]
            for sg in range(8):
                slab = load_slab(xT_d, sg * 512, 512)
                for ss in range(4):
                    for h in range(2):
                        ps = psmm.tile([P, 512], f32, tag="psmm", name="ps")
                        for d in range(DK):
                            nc.tensor.matmul(
                                ps[:],
                                slab[d][:, ss * P : (ss + 1) * P],
                                wv[d][:, h * 512 : (h + 1) * 512],
                                start=(d == 0),
                                stop=(d == DK - 1),
                            )
                        nc.vector.tensor_copy(
                            v[sg * 4 + ss][:, h * 512 : (h + 1) * 512], ps[:]
                        )

            # ---- attention, software-pipelined by one query block
            state = None
            for t in range(NQB + 1):
                new_state = None
                if t < NQB:
                    nsb = 2 * t + 2          # 128-col key blocks in extent
                    ext = nsb * P
                    G = math.ceil(ext / 512)  # 512-col score groups
                    sums = statp.tile([P, G], f32, tag="sums", name="sums")
                    attnT = attnTp.tile([P, ext], bf16, tag="attnT", name="attnT")
                    for g in range(G):
                        gw = min(512, ext - g * 512)
                        ps = psmm.tile([P, gw], f32, tag="psmm", name="ps")
                        for o in range(DK):
                            nc.tensor.matmul(
                                ps[:],
                                qT[o][:, t * P : (t + 1) * P],
                                kT[o][:, g * 512 : g * 512 + gw],
                                start=(o == 0),
                                stop=(o == DK - 1),
                            )
                        if g == G - 1:
                            nc.vector.tensor_add(
                                ps[:, gw - 2 * P : gw],
                                ps[:, gw - 2 * P : gw],
                                mask_t[:],
                            )
                        attn_g = attnp.tile([P, gw], bf16, tag="attn", name="attn_g")
                        nc.scalar.activation(
                            attn_g[:],
                            ps[:],
                            mybir.ActivationFunctionType.Exp,
                            scale=1.0 / 32.0,
                            accum_out=sums[:, g : g + 1],
                        )
                        for jj in range(gw // P):
                            j = g * 4 + jj
                            pst = pstr.tile([P, P], bf16, tag="pstr", name="pst")
                            nc.tensor.transpose(
                                pst[:], attn_g[:, jj * P : (jj + 1) * P], ident_t[:]
                            )
                            nc.vector.tensor_copy(
                                attnT[:, j * P : (j + 1) * P], pst[:]
                            )
                    tot = statp.tile([P, 1], f32, tag="tot", name="tot")
                    nc.vector.reduce_sum(tot[:], sums[:], axis=mybir.AxisListType.X)
                    recip = statp.tile([P, 1], f32, tag="recip", name="recip")
                    nc.vector.reciprocal(recip[:], tot[:])
                    new_state = (attnT, recip, nsb, t)

                if state is not None:
                    attnT_p, recip_p, nsb_p, tp = state
                    outst = outp.tile([P, D], f32, tag="outst", name="outst")
                    for h in range(2):
                        ps = pspv.tile([P, 512], f32, tag="pspv", name="pspv")
                        for j in range(nsb_p):
                            nc.tensor.matmul(
                                ps[:],
                                attnT_p[:, j * P : (j + 1) * P],
                                v[j][:, h * 512 : (h + 1) * 512],
                                start=(j == 0),
                                stop=(j == nsb_p - 1),
                            )
                        nc.vector.tensor_scalar_mul(
                            outst[:, h * 512 : (h + 1) * 512], ps[:], recip_p[:]
                        )
                    nc.sync.dma_start(out_d[tp], outst[:])

                state = new_state

    nc.compile()
    return nc


def _get_program():
    if "nc" not in _CACHE:
        _CACHE["nc"] = _build_program()
    return _CACHE["nc"]


def _make_in_maps(x, Wq, Wk, Wv):
    bf16 = ml_dtypes.bfloat16
    wqT = np.ascontiguousarray(Wq.T).astype(bf16).reshape(DK, P, D)
    wkT = np.ascontiguousarray(Wk.T).astype(bf16).reshape(DK, P, D)
    wvT = np.ascontiguousarray(Wv.T).astype(bf16).reshape(DK, P, D)
    ident = np.eye(P, dtype=np.float32).astype(bf16)

    masks = []
    tri = np.triu(np.full((P, P), NEG, np.float32), k=1)  # [i,j]=NEG where j>i
    for k in range(2):
        m = np.zeros((P, 2 * P), np.float32)
        if k == 0:
            m[:, :P] = tri
            m[:, P:] = NEG
        else:
            m[:, P:] = tri
        masks.append(m)

    in_maps = []
    for c in range(NCORES):
        b, k = c // 2, c % 2
        xb_T = np.ascontiguousarray(x[b].T.astype(bf16))  # [D, S]
        q_cols = np.concatenate(
            [np.arange((2 * t + k) * P, (2 * t + k + 1) * P) for t in range(NQB)]
        )
        xTq = np.ascontiguousarray(xb_T[:, q_cols])
        in_maps.append(
            {
                "xT": xb_T.reshape(DK, P, S),
                "xTq": xTq.reshape(DK, P, S // 2),
                "wqT": wqT,
                "wkT": wkT,
                "wvT": wvT,
                "mask": masks[k],
                "ident": ident,
            }
        )
    return in_maps


def kernel(x, Wq, Wk, Wv):
    x = np.asarray(x, dtype=np.float32)
    Wq = np.asarray(Wq, dtype=np.float32)
    Wk = np.asarray(Wk, dtype=np.float32)
    Wv = np.asarray(Wv, dtype=np.float32)

    nc = _get_program()
    in_maps = _make_in_maps(x, Wq, Wk, Wv)
    res = run_bass_kernel_spmd(nc, in_maps, list(range(NCORES)))

    out = np.empty((B, S, D), np.float32)
    for c in range(NCORES):
        b, k = c // 2, c % 2
        oc = res.results[c]["out"]  # [NQB, P, D]
        for t in range(NQB):
            g = 2 * t + k
            out[b, g * P : (g + 1) * P, :] = oc[t]
    return out
